# revision 17
# baseline (speedup 1.0000x reference)
"""Dual (global + local-masked) BERT self-attention on 8 Trainium2 NeuronCores.

Problem: B=2, S=2048, H=1024, NH=16 heads of DH=64.
  q/k/v = hidden @ W{q,k,v}.T + b ; scores = q k^T / 8
  probs_g = softmax(scores + attention_mask)         (additive, zeros in spec)
  probs_l = softmax(scores + (-inf where local_mask==0))
  out     = gate * (probs_l @ v) + (1-gate) * (probs_g @ v)

Sharding: 32 (batch, head) pairs -> 4 heads per core (core c: batch c//4,
heads 4*(c%4)..+4). Each core computes its heads' projections + dual
attention independently; no collectives.

Per-core kernel (all layouts transposed so softmax reductions ride the
TensorEngine):
  - X^T, W slices, local mask (as 0/1 bf16, transposed) are DMA'd in.
  - Q^T,K^T [128dims(2-head pair), S] and natural-layout V (+ ones column)
    are computed on PE in bf16.
  - scores^T tile [128 keys, 512 q] per (pair, k-tile): two row-packed
    64-contraction matmuls (head A on array rows 0-63, head B on 64-127).
  - e = exp(scores) once on ACT (shared by both branches), e_l = e * mask
    on DVE.
  - ctx^T accumulated over k-tiles: lhsT = [V_tile | ones] so row 64 of the
    PSUM accumulates the softmax denominator for free.
  - epilogue: recip(sums) * gate rows -> per-q coefficients, broadcast to
    64 partitions via tiny K=1 matmuls, combine on DVE, DMA out.
Output per core: [256 dims, 2048 q] f32; host transposes/reassembles.
"""

import sys

sys.path.insert(0, "/opt/trn_rl_repo")

import numpy as np
import ml_dtypes

B, S, H, NH, DH = 2, 2048, 1024, 16, 64
NCORES = 8
HPC = 4          # heads per core
MPC = HPC // 2   # head pairs per core
QC = 512         # query chunk (free dim of scores/ctx psums)
NQC = S // QC
KT = S // 128    # key tiles
XT_T = H // 128  # X^T k-tiles for projections

_BUILT = {}


def _build(use_em: bool):
    from contextlib import ExitStack

    import concourse.mybir as mybir
    from concourse import bacc, tile

    f32 = mybir.dt.float32
    bf16 = mybir.dt.bfloat16
    AF = mybir.ActivationFunctionType

    nc = bacc.Bacc("TRN2", target_bir_lowering=False, debug=False)

    xt_d = nc.dram_tensor("xt", [H, S], bf16, kind="ExternalInput").ap()
    wq_d = nc.dram_tensor("wq", [H, 256], bf16, kind="ExternalInput").ap()
    wk_d = nc.dram_tensor("wk", [H, 256], bf16, kind="ExternalInput").ap()
    wv_d = nc.dram_tensor("wv", [H, 256], bf16, kind="ExternalInput").ap()
    bqk_d = nc.dram_tensor("bqk", [2, 256], f32, kind="ExternalInput").ap()
    bv_d = nc.dram_tensor("bv", [1, 256], bf16, kind="ExternalInput").ap()
    msk_d = nc.dram_tensor("msk", [KT, 128, S], bf16, kind="ExternalInput").ap()
    # gt[r, m, q]: pair m, r = (gate_A, 1-gate_A, gate_B, 1-gate_B)
    gt_d = nc.dram_tensor("gt", [4, MPC, S], f32, kind="ExternalInput").ap()
    # sel[r, j, d] = 1.0 if r == j else 0 — K=4 broadcast selectors
    sel_d = nc.dram_tensor("sel", [4, 4, 64], f32, kind="ExternalInput").ap()
    if use_em:
        em_d = nc.dram_tensor("em", [KT, 128], f32, kind="ExternalInput").ap()
    out_d = nc.dram_tensor("out", [HPC * DH, S], f32, kind="ExternalOutput").ap()

    with tile.TileContext(nc) as tc, ExitStack() as ctx:
        big = ctx.enter_context(tc.tile_pool(name="big", bufs=1))

        xt_sb = big.tile([128, XT_T, S], bf16, name="xt_sb")
        for t in range(XT_T):
            nc.sync.dma_start(xt_sb[:, t, :], xt_d[t * 128:(t + 1) * 128, :])
        w_sbs = {}
        for nm, d in (("wq", wq_d), ("wk", wk_d), ("wv", wv_d)):
            w_sb = big.tile([128, XT_T, 256], bf16, name=f"{nm}_sb")
            for t in range(XT_T):
                nc.sync.dma_start(w_sb[:, t, :], d[t * 128:(t + 1) * 128, :])
            w_sbs[nm] = w_sb
        msk_sb = big.tile([128, KT, S], bf16, name="msk_sb")
        for t in range(KT):
            nc.sync.dma_start(msk_sb[:, t, :], msk_d[t])
        gt_sb = big.tile([4, MPC, S], f32, name="gt_sb")
        nc.sync.dma_start(gt_sb, gt_d)
        sel_sb = big.tile([4, 4, 64], f32, name="sel_sb")
        nc.sync.dma_start(sel_sb, sel_d)
        bqk_sb = big.tile([128, 2, 2], f32, name="bqk_sb")
        nc.sync.dma_start(
            bqk_sb, bqk_d.rearrange("c (t p) -> p c t", p=128)
        )
        bv_sb = big.tile([1, 256], bf16, name="bv_sb")
        nc.sync.dma_start(bv_sb, bv_d)
        if use_em:
            em_sb = big.tile([128, KT], f32, name="em_sb")
            nc.sync.dma_start(em_sb, em_d.rearrange("t p -> p t"))

        ones_r = big.tile([1, 128], bf16, name="ones_r")
        nc.vector.memset(ones_r, 1.0)

        qt_sb = big.tile([128, MPC, S], bf16, name="qt_sb")
        kt_sb = big.tile([128, MPC, S], bf16, name="kt_sb")
        v_sb = big.tile([128, KT, HPC, 65], bf16, name="v_sb")
        nc.vector.memset(v_sb[:, :, :, 64:65], 1.0)

        # ---- projections: Q^T, K^T (transposed), V (natural) ----
        with tc.tile_pool(name="pproj", bufs=2, space="PSUM") as pproj:
            for m in range(MPC):
                for ci, (wn, dst) in enumerate((("wq", qt_sb), ("wk", kt_sb))):
                    w_sb = w_sbs[wn]
                    for nq in range(S // 512):
                        ps = pproj.tile([128, 512], f32, tag="pp")
                        for t in range(XT_T):
                            nc.tensor.matmul(
                                ps,
                                lhsT=w_sb[:, t, m * 128:(m + 1) * 128],
                                rhs=xt_sb[:, t, nq * 512:(nq + 1) * 512],
                                start=(t == 0),
                                stop=(t == XT_T - 1),
                            )
                        nc.scalar.activation(
                            dst[:, m, nq * 512:(nq + 1) * 512], ps,
                            AF.Identity, bias=bqk_sb[:, ci, m:m + 1], scale=1.0,
                        )
        with tc.tile_pool(name="pv", bufs=2, space="PSUM") as pv:
            for st in range(KT):
                ps = pv.tile([128, 256], f32, tag="pv")
                for t in range(XT_T):
                    nc.tensor.matmul(
                        ps,
                        lhsT=xt_sb[:, t, st * 128:(st + 1) * 128],
                        rhs=w_sbs["wv"][:, t, :],
                        start=(t == 0),
                        stop=False,
                    )
                nc.tensor.matmul(
                    ps, lhsT=ones_r, rhs=bv_sb, start=False, stop=True
                )
                nc.scalar.activation(
                    v_sb[:, st, :, 0:64],
                    ps.rearrange("p (h d) -> p h d", h=HPC),
                    AF.Copy,
                )

        # ---- dual attention ----
        psc = ctx.enter_context(tc.tile_pool(name="psc", bufs=4, space="PSUM"))
        pctx = ctx.enter_context(tc.tile_pool(name="pctx", bufs=1, space="PSUM"))  # 4 tags x 1 = 4 banks
        pe = ctx.enter_context(tc.tile_pool(name="pe", bufs=8))
        pt = ctx.enter_context(tc.tile_pool(name="pt", bufs=4))
        po = ctx.enter_context(tc.tile_pool(name="po", bufs=4))
        pc = ctx.enter_context(tc.tile_pool(name="pc", bufs=8))

        for m in range(MPC):
            for qc in range(NQC):
                qs = slice(qc * QC, (qc + 1) * QC)
                cps = [
                    pctx.tile([65, QC], f32, name=f"ctx{j}", tag=f"ctx{j}")
                    for j in range(4)
                ]  # order: Ag, Al, Bg, Bl
                for t in range(KT):
                    st0 = (t == 0)
                    st1 = (t == KT - 1)
                    psA = psc.tile([128, QC], f32, name="psA", tag="sc")
                    psB = psc.tile([128, QC], f32, name="psB", tag="sc")
                    nc.tensor.matmul(
                        psA,
                        lhsT=kt_sb[0:64, m, t * 128:(t + 1) * 128],
                        rhs=qt_sb[0:64, m, qs],
                        start=True, stop=True,
                    )
                    nc.tensor.matmul(
                        psB,
                        lhsT=kt_sb[64:128, m, t * 128:(t + 1) * 128],
                        rhs=qt_sb[64:128, m, qs],
                        start=True, stop=True,
                    )
                    eA = pe.tile([128, QC], bf16, name="eA", tag="e")
                    eB = pe.tile([128, QC], bf16, name="eB", tag="e")
                    nc.scalar.activation(eA, psA, AF.Exp)
                    nc.scalar.activation(eB, psB, AF.Exp)
                    elA = pe.tile([128, QC], bf16, name="elA", tag="e")
                    elB = pe.tile([128, QC], bf16, name="elB", tag="e")
                    nc.vector.tensor_mul(elA, eA, msk_sb[:, t, qs])
                    nc.vector.tensor_mul(elB, eB, msk_sb[:, t, qs])
                    if use_em:
                        egA = pe.tile([128, QC], bf16, name="egA", tag="e")
                        egB = pe.tile([128, QC], bf16, name="egB", tag="e")
                        nc.vector.tensor_scalar_mul(egA, eA, em_sb[:, t:t + 1])
                        nc.vector.tensor_scalar_mul(egB, eB, em_sb[:, t:t + 1])
                    else:
                        egA, egB = eA, eB
                    for j, ee in ((0, egA), (1, elA), (2, egB), (3, elB)):
                        nc.tensor.matmul(
                            cps[j],
                            lhsT=v_sb[:, t, 2 * m + j // 2, :],
                            rhs=ee,
                            start=st0, stop=st1,
                        )
                # epilogue: sums (psum row 64) -> [4, QC] at base partition 0,
                # recip * gate, broadcast via K=4 selector matmuls, combine.
                # stage rows: (l_A, g_A, l_B, g_B); cps order (Ag, Al, Bg, Bl)
                stage = pc.tile([65, 4, QC], f32, name="stage", tag="stage", bufs=2)
                for j, src in enumerate((cps[1], cps[0], cps[3], cps[2])):
                    nc.scalar.activation(stage[64:65, j, :], src[64:65, :], AF.Copy)
                sums4 = pc.tile([4, QC], f32, name="sums4", tag="sums", bufs=4)
                nc.sync.dma_start(sums4, stage[64:65, :, :])
                rec4 = pc.tile([4, QC], f32, name="rec4", tag="sums", bufs=4)
                nc.vector.reciprocal_approx_fast(rec4, sums4)
                coef4 = pc.tile([4, QC], f32, name="coef4", tag="sums", bufs=4)
                nc.vector.tensor_mul(coef4, rec4, gt_sb[:, m, qs])
                for jj in range(2):
                    h = 2 * m + jj
                    ctg, ctl = cps[2 * jj], cps[2 * jj + 1]
                    bcl = psc.tile([64, QC], f32, name="bcl", tag="sc")
                    bcg = psc.tile([64, QC], f32, name="bcg", tag="sc")
                    nc.tensor.matmul(bcl, lhsT=sel_sb[:, 2 * jj, :], rhs=coef4,
                                     start=True, stop=True)
                    nc.tensor.matmul(bcg, lhsT=sel_sb[:, 2 * jj + 1, :], rhs=coef4,
                                     start=True, stop=True)
                    bcl_s = pt.tile([64, QC], f32, name="bcl_s", tag="bc")
                    bcg_s = pt.tile([64, QC], f32, name="bcg_s", tag="bc")
                    nc.scalar.activation(bcl_s, bcl, AF.Copy)
                    nc.scalar.activation(bcg_s, bcg, AF.Copy)
                    t1 = pt.tile([64, QC], f32, name="t1", tag="t")
                    t2 = pt.tile([64, QC], f32, name="t2", tag="t")
                    nc.vector.tensor_mul(t1, ctl[0:64, :], bcl_s)
                    nc.vector.tensor_mul(t2, ctg[0:64, :], bcg_s)
                    o = po.tile([64, QC], f32, name="o", tag="o")
                    nc.vector.tensor_add(o, t1, t2)
                    nc.sync.dma_start(out_d[h * 64:(h + 1) * 64, qs], o)

    nc.compile()
    return nc


def _get(use_em: bool):
    if use_em not in _BUILT:
        _BUILT[use_em] = _build(use_em)
    return _BUILT[use_em]


def _prep_core(c, hs, am, lm, go, Wq, bq, Wk, bk, Wv, bv, use_em):
    bf = ml_dtypes.bfloat16
    b, hg = c // 4, c % 4
    h0 = hg * HPC
    sl = slice(h0 * DH, (h0 + HPC) * DH)
    m = {
        "xt": np.ascontiguousarray(hs[b].T).astype(bf),
        "wq": np.ascontiguousarray((Wq[sl, :] / 8.0).T).astype(bf),
        "wk": np.ascontiguousarray(Wk[sl, :].T).astype(bf),
        "wv": np.ascontiguousarray(Wv[sl, :].T).astype(bf),
        "bqk": np.stack([bq[sl] / 8.0, bk[sl]]).astype(np.float32),
        "bv": bv[sl].reshape(1, 256).astype(bf),
        "msk": np.ascontiguousarray(
            lm[b, 0].astype(np.float32).T).reshape(KT, 128, S).astype(bf),
        "gt": np.stack([
            np.stack([go[b, h0 + 2 * m + (r // 2), :, 0] if r % 2 == 0
                      else 1.0 - go[b, h0 + 2 * m + (r // 2), :, 0]
                      for m in range(MPC)])
            for r in range(4)]).astype(np.float32),
        "sel": np.broadcast_to(
            np.eye(4, dtype=np.float32)[:, :, None], (4, 4, 64)).copy(),
    }
    if use_em:
        m["em"] = np.exp(am[b, 0, 0]).astype(np.float32).reshape(KT, 128)
    return m


def make_in_maps(inputs):
    hs = np.asarray(inputs["hidden_states"], np.float32)
    am = np.asarray(inputs["attention_mask"], np.float32)
    lm = np.asarray(inputs["local_attention_mask"])
    go = np.asarray(inputs["gate_outputs"], np.float32)
    Wq = np.asarray(inputs["Wq"], np.float32)
    bq = np.asarray(inputs["bq"], np.float32)
    Wk = np.asarray(inputs["Wk"], np.float32)
    bk = np.asarray(inputs["bk"], np.float32)
    Wv = np.asarray(inputs["Wv"], np.float32)
    bv = np.asarray(inputs["bv"], np.float32)
    use_em = bool(np.any(am != 0.0))
    maps = [
        _prep_core(c, hs, am, lm, go, Wq, bq, Wk, bk, Wv, bv, use_em)
        for c in range(NCORES)
    ]
    return maps, use_em


def assemble(results):
    out = np.empty((B, S, H), np.float32)
    for c in range(NCORES):
        b, hg = c // 4, c % 4
        sl = slice(hg * HPC * DH, (hg + 1) * HPC * DH)
        out[b, :, sl] = np.asarray(results[c]["out"]).T
    return out


def kernel(**inputs):
    from concourse import bass_utils

    maps, use_em = make_in_maps(inputs)
    nc = _get(use_em)
    res = bass_utils.run_bass_kernel_spmd(nc, maps, core_ids=list(range(NCORES)))
    return assemble(res.results)
